# revision 26
# baseline (speedup 1.0000x reference)
"""Causal self-attention (B=2, T=4096, C=768, H=12) on 8 trn2 NeuronCores.

Sharding: core c -> batch b = c//4, head group g = c%4 (3 heads per core).
Each core computes QKV projection for its 3 heads, causal attention, and a
partial output projection (its heads' contribution to y @ w_proj, transposed).
Host sums the 4 partials per batch and adds b_proj.

On-device layout (per core):
  - x^T (C, T) fp8e4 is the only activation input; QKV runs in fp8 DoubleRow
    perf mode (2 contraction k-tiles per pass, 2x PE throughput).  Weights are
    host-prescaled by 16 so fp8 quantization stays in the normal range; the
    1/16 is folded into the PSUM-eviction scale (Q/K) or cancels between the
    V rows and the ones-rows of V' (the softmax denominator).
  - Q^T/K^T (64, T) bf16 per head come straight out of W-stationary matmuls.
  - Attention runs in the S^T orientation: S^T[k, q] = K^T.T @ Q^T tiles, so
    softmax exp runs on ScalarE with no max pass and no P transpose.  The
    causal mask is a multiplicative post-exp mask on the Pool engine.
  - The softmax denominator comes from a ones-column appended to V:
    Y'[0:64] = unnormalized Y^T, Y'[64] = row sums l.  1/l comes from the
    fast custom-DVE reciprocal, is broadcast to 64 partitions with a K=1
    f32r matmul, and applied with one VectorE multiply.
  - Output projection packs heads 0+1 on 128 partitions (one K=128 matmul)
    plus head 2 as K=64, emitting partial^T (C, T) f32.  PSUM evictions run
    on the otherwise-idle Pool engine.
"""

import os
import sys

import numpy as np

for _p in ("/opt/trn_rl_repo", "/root/.axon_site/_ro/trn_rl_repo"):
    if os.path.isdir(_p) and _p not in sys.path:
        sys.path.insert(0, _p)

import ml_dtypes

import concourse.bacc as bacc
import concourse.bass as bass
import concourse.mybir as mybir
import concourse.tile as tile
from concourse.bass_utils import run_bass_kernel_spmd

B, T, C = 2, 4096, 768
H, HD = 12, 64
NCORES = 8
HPC = 3  # heads per core
P = 128
NBLK = T // 512  # 8 q-blocks of 512
NKT = T // 128  # 32 k-tiles of 128
KC = C // 128  # 6 contraction chunks (3 DoubleRow pairs)
USE_FP8 = os.environ.get("KERNEL_NO_FP8", "") != "1"
WS = 16.0 if USE_FP8 else 1.0  # host prescale on fp8 weights

F32 = mybir.dt.float32
F32R = mybir.dt.float32r
BF16 = mybir.dt.bfloat16
FP8 = mybir.dt.float8e4
BF16_NP = ml_dtypes.bfloat16
FP8_NP = ml_dtypes.float8_e4m3
AF = mybir.ActivationFunctionType
DR = mybir.MatmulPerfMode.DoubleRow

_CACHE = {}


def _build_nc():
    nc = bacc.Bacc("TRN2", target_bir_lowering=False, debug=False)

    XDT = FP8 if USE_FP8 else BF16
    xt_d = nc.dram_tensor("xt", [C, T], XDT, kind="ExternalInput")
    wqk_d = nc.dram_tensor("wqk", [C, 512], XDT, kind="ExternalInput")
    wv_d = nc.dram_tensor("wv", [C, HPC * HD], XDT, kind="ExternalInput")
    wp_d = nc.dram_tensor("wp", [HPC, HD, C], BF16, kind="ExternalInput")
    scale_d = nc.dram_tensor("scale_qk", [P, 4], F32, kind="ExternalInput")
    bias_d = nc.dram_tensor("bias_qk", [P, 4], F32, kind="ExternalInput")
    bv_d = nc.dram_tensor("bv", [P, HPC * HD], F32, kind="ExternalInput")
    mask_d = nc.dram_tensor("mask", [P, P], BF16, kind="ExternalInput")
    out_d = nc.dram_tensor("outT", [C, T], F32, kind="ExternalOutput")

    with tile.TileContext(nc) as tc:
        with (
            tc.tile_pool(name="store", bufs=1) as store,
            tc.tile_pool(name="consts", bufs=1) as consts,
            tc.tile_pool(name="pt_pool", bufs=5) as pt_pool,
            tc.tile_pool(name="yt_pool", bufs=4) as yt_pool,
            tc.tile_pool(name="rsb_pool", bufs=4) as rsb_pool,
            tc.tile_pool(name="osb_pool", bufs=3) as osb_pool,
            tc.tile_pool(name="s_psum", bufs=2, space="PSUM") as s_psum,
            tc.tile_pool(name="y_psum", bufs=2, space="PSUM") as y_psum,
            tc.tile_pool(name="m_psum", bufs=2, space="PSUM") as m_psum,
        ):
            # ---- persistent SBUF storage ----
            XT = store.tile([P, KC, T], XDT)  # x^T, 6 chunks of 128 rows
            WQK = store.tile([P, KC, 512], XDT)
            WV = store.tile([P, KC, HPC * HD], XDT)
            WP01 = store.tile([P, C], BF16)  # w_proj rows: h0 on 0:64, h1 on 64:128
            WP2 = store.tile([HD, C], BF16)
            QT01 = store.tile([P, T], BF16)  # Q^T h0 @0-63, h1 @64-127
            KT01 = store.tile([P, T], BF16)
            QT2 = store.tile([HD, T], BF16)
            KT2 = store.tile([HD, T], BF16)
            VN = store.tile([P, NKT, HPC, HD + 2], BF16)  # V' with 2 ones cols
            YN01 = store.tile([P, T], BF16)  # normalized Y^T: h0 @0-63, h1 @64-127
            YN2 = store.tile([HD, T], BF16)

            scale_qk = consts.tile([P, 4], F32)
            bias_qk = consts.tile([P, 4], F32)
            bvb = consts.tile([P, HPC * HD], F32)  # host-prebroadcast bias (x16)
            mask = consts.tile([P, P], BF16)
            # r broadcast operands: hi row at partition 0, lo row at partition
            # 32 (legal AP bases), zero rows between; 4-slot ring in free dim
            onesb2 = consts.tile([33, HD], BF16)
            RHL = store.tile([33, 4, 512], BF16)

            # ---- input DMAs: weights + first token chunk first so the
            # prologue's first matmul isn't gated on the full x^T transfer ----
            xt_view = xt_d.rearrange("(k p) t -> p k t", p=P)
            nc.sync.dma_start(WQK[:], wqk_d.rearrange("(k p) c -> p k c", p=P))
            nc.sync.dma_start(scale_qk[:], scale_d[:])
            nc.sync.dma_start(bias_qk[:], bias_d[:])
            nc.sync.dma_start(
                XT[:, :, 0:512],
                xt_view[:, :, 0:512],
            )
            nc.sync.dma_start(WV[:], wv_d.rearrange("(k p) c -> p k c", p=P))
            nc.sync.dma_start(bvb[:], bv_d[:])
            nc.sync.dma_start(mask[:], mask_d[:])
            for n in range(1, NBLK):
                nc.sync.dma_start(
                    XT[:, :, n * 512 : (n + 1) * 512],
                    xt_view[:, :, n * 512 : (n + 1) * 512],
                )
            nc.sync.dma_start(WP01[:], wp_d[0:2].rearrange("h p c -> (h p) c"))
            nc.sync.dma_start(WP2[:], wp_d[2])

            # V rows carry the x16 weight prescale, so the ones-rows carry it
            # too and the scale cancels in Y'/l.
            nc.any.memset(VN[:, :, :, HD : HD + 2], WS)
            nc.any.memset(onesb2[:], 0.0)
            nc.any.memset(onesb2[0:1, :], 1.0)
            nc.any.memset(onesb2[32:33, :], 1.0)
            nc.any.memset(RHL[:], 0.0)

            # fp8 DoubleRow k-tile-pair views of the QKV operands
            XTp = XT[:].rearrange("p (a b) t -> p a b t", b=2)
            WQKp = WQK[:].rearrange("p (a b) c -> p a b c", b=2)
            WVp = WV[:].rearrange("p (a b) c -> p a b c", b=2)

            # ---- work-group builders ----
            # M-tiles of wqk cols: 0=[qh0;qh1] 1=[kh0;kh1] 2=[qh2;pad] 3=[kh2;pad]
            def qkv_group(m, n):
                ps = m_psum.tile([P, 512], F32, tag="misc")
                if USE_FP8:
                    for k in range(KC // 2):
                        nc.tensor.matmul(
                            ps[:],
                            WQKp[:, k, :, m * P : (m + 1) * P],
                            XTp[:, k, :, n * 512 : (n + 1) * 512],
                            start=(k == 0),
                            stop=(k == KC // 2 - 1),
                            perf_mode=DR,
                        )
                else:
                    for k in range(KC):
                        nc.tensor.matmul(
                            ps[:],
                            WQK[:, k, m * P : (m + 1) * P],
                            XT[:, k, n * 512 : (n + 1) * 512],
                            start=(k == 0),
                            stop=(k == KC - 1),
                        )
                dst = (QT01, KT01, QT2, KT2)[m]
                rows = P if m < 2 else HD
                nc.vector.tensor_scalar(
                    dst[0:rows, n * 512 : (n + 1) * 512],
                    ps[0:rows, :],
                    scale_qk[0:rows, m : m + 1],
                    bias_qk[0:rows, m : m + 1],
                    op0=mybir.AluOpType.mult,
                    op1=mybir.AluOpType.add,
                )

            def v_group(mt):
                vp = m_psum.tile([P, HPC * HD], F32, tag="misc")
                if USE_FP8:
                    for k in range(KC // 2):
                        nc.tensor.matmul(
                            vp[:],
                            XTp[:, k, :, mt * P : (mt + 1) * P],
                            WVp[:, k, :, :],
                            start=(k == 0),
                            stop=(k == KC // 2 - 1),
                            perf_mode=DR,
                        )
                else:
                    for k in range(KC):
                        nc.tensor.matmul(
                            vp[:],
                            XT[:, k, mt * P : (mt + 1) * P],
                            WV[:, k, :],
                            start=(k == 0),
                            stop=(k == KC - 1),
                        )
                # bias folded into the PSUM->SBUF eviction
                nc.vector.tensor_add(
                    VN[:, mt, :, 0:HD],
                    vp[:].rearrange("p (h d) -> p h d", h=HPC),
                    bvb[:].rearrange("p (h d) -> p h d", h=HPC),
                )

            def proj_group(m, n):
                ops = m_psum.tile([P, 512], F32, tag="misc")
                nc.tensor.matmul(
                    ops[:],
                    WP01[:, m * P : (m + 1) * P],
                    YN01[:, n * 512 : (n + 1) * 512],
                    start=True,
                    stop=False,
                )
                nc.tensor.matmul(
                    ops[:],
                    WP2[:, m * P : (m + 1) * P],
                    YN2[:, n * 512 : (n + 1) * 512],
                    start=False,
                    stop=True,
                )
                osb = osb_pool.tile([P, 512], F32)
                nc.vector.tensor_copy(osb[:], ops[:])
                nc.sync.dma_start(
                    out_d[m * P : (m + 1) * P, n * 512 : (n + 1) * 512],
                    osb[:],
                )

            # ---- filler queue: PE work drip-fed into the attention phase so
            # the tensor engine never micro-idles ----
            from collections import deque

            filler_q = deque()
            chunk_done = [2]  # chunks fully emitted (prologue: 0-2)

            def pop_filler(k):
                for _ in range(k):
                    if not filler_q:
                        return
                    n_final, fn = filler_q.popleft()
                    fn()
                    if n_final is not None:
                        chunk_done[0] = max(chunk_done[0], n_final)

            def drain_through_chunk(n):
                while filler_q and chunk_done[0] < n:
                    pop_filler(1)

            # ---- attention head-block (S^T orientation, LAG-pipelined) ----
            # (QT, KT, row offset, YN destination row offset or YN2)
            heads = (
                (QT01, KT01, 0, YN01, 0),
                (QT01, KT01, HD, YN01, HD),
                (QT2, KT2, 0, YN2, 0),
            )
            LAG = 2  # PV trails S by LAG exp-chunks

            def emit_pv(h, i, yps, ent):
                pt, off0, off1, j0, j1 = ent
                jlast = 4 * i + 3
                nc.tensor.matmul(
                    yps[:, off0:],
                    VN[:, j0, h, :],
                    pt[:, off0:512],
                    start=(j0 == 0),
                    stop=False,
                )
                nc.tensor.matmul(
                    yps[:, off1:],
                    VN[:, j1, h, :],
                    pt[:, 512 + off1 : 1024],
                    start=False,
                    stop=(j1 == jlast),
                )

            deferred = []  # (PE bcast + mul) closures from prior blocks

            def flush_norms():
                while deferred:
                    deferred.pop(0)()

            def attn_block(h, i):
                QTt, KTt, b0, YNd, r0 = heads[h]
                yps = y_psum.tile([HD + 2, 512], F32)
                pending = []
                for c in range(2 * i + 2):
                    j0, j1 = 2 * c, 2 * c + 1
                    off0 = max(0, j0 - 4 * i) * P
                    off1 = max(0, j1 - 4 * i) * P
                    sps = s_psum.tile([P, 1024], F32)
                    nc.tensor.matmul(
                        sps[:, off0:512],
                        KTt[b0 : b0 + HD, j0 * P : (j0 + 1) * P],
                        QTt[b0 : b0 + HD, i * 512 + off0 : (i + 1) * 512],
                        start=True,
                        stop=True,
                    )
                    nc.tensor.matmul(
                        sps[:, 512 + off1 : 1024],
                        KTt[b0 : b0 + HD, j1 * P : (j1 + 1) * P],
                        QTt[b0 : b0 + HD, i * 512 + off1 : (i + 1) * 512],
                        start=True,
                        stop=True,
                    )
                    pt = pt_pool.tile([P, 1024], BF16)
                    # single exp per chunk: the start=True matmuls zero their
                    # whole 2KB PSUM bank, so the diagonal gap reads exp(0),
                    # which PV never consumes (CoreSim flags the gap read as
                    # uninitialized; set KERNEL_SPLIT_EXP=1 for sim runs)
                    if off1 > off0 and os.environ.get("KERNEL_SPLIT_EXP") == "1":
                        nc.scalar.activation(
                            pt[:, off0:512], sps[:, off0:512], AF.Exp
                        )
                        nc.scalar.activation(
                            pt[:, 512 + off1 :], sps[:, 512 + off1 :], AF.Exp
                        )
                    else:
                        nc.scalar.activation(pt[:, off0:], sps[:, off0:], AF.Exp)
                    # causal mask: zero the upper-k triangle of diagonal units
                    # (multiplicative, post-exp, on the idle Pool engine;
                    # SBUF->SBUF so Pool is legal)
                    if j0 >= 4 * i:
                        nc.gpsimd.tensor_mul(
                            pt[:, off0 : off0 + P], pt[:, off0 : off0 + P], mask[:]
                        )
                    if j1 >= 4 * i:
                        nc.gpsimd.tensor_mul(
                            pt[:, 512 + off1 : 512 + off1 + P],
                            pt[:, 512 + off1 : 512 + off1 + P],
                            mask[:],
                        )
                    pending.append((pt, off0, off1, j0, j1))
                    if len(pending) > LAG:
                        emit_pv(h, i, yps, pending.pop(0))
                    if c == 2 and len(deferred) >= 2:
                        deferred.pop(0)()
                    if c % 6 == 5:
                        pop_filler(1)
                # Filler BEFORE the pending-PV drain: the lagged PVs wait on
                # the last exps, which have no S-work left to hide behind.
                pop_filler(1 + (i + 1) // 2)
                while pending:
                    emit_pv(h, i, yps, pending.pop(0))
                # normalize: r = 1/l via fast custom-DVE reciprocal (l > 0),
                # split into bf16 hi/lo rows of one [2, 512] tile, then a
                # deferred single K=2 broadcast matmul + one VectorE multiply
                # -- full precision at K=1 matmul cost.
                # stage l into SBUF first: the custom-DVE fast reciprocal
                # reads garbage from PSUM inputs on hardware
                lsb = rsb_pool.tile([1, 512], F32, tag="lsb")
                nc.vector.tensor_copy(lsb[:], yps[HD : HD + 1, :])
                rsb = rsb_pool.tile([1, 512], F32, tag="rsb")
                nc.vector.reciprocal_approx_fast(rsb[:], lsb[:])
                yt = yt_pool.tile([HD, 512], BF16)
                nc.vector.tensor_copy(yt[:], yps[0:HD, :])
                slot = (HPC * i + h) % 4
                nc.vector.tensor_copy(RHL[0:1, slot, :], rsb[:])
                nc.vector.tensor_sub(RHL[32:33, slot, :], rsb[:], RHL[0:1, slot, :])

                def bcast_mul(i=i, yt=yt, slot=slot, YNd=YNd, r0=r0):
                    rps = m_psum.tile([HD, 512], F32, tag="misc")
                    nc.tensor.matmul(
                        rps[:], onesb2[:], RHL[:, slot, :], start=True, stop=True
                    )
                    nc.vector.tensor_mul(
                        YNd[r0 : r0 + HD, i * 512 : (i + 1) * 512], yt[:], rps[:]
                    )

                # defer the PE broadcast into the next head-block so the PE
                # never waits for the DVE reciprocal at the boundary
                deferred.append(bcast_mul)

            # ---- prologue: QKV + V for token chunks 0-2 (dense PE warmup) ----
            for n in range(3):
                for m in range(4):
                    qkv_group(m, n)
                    v_group(4 * n + m)

            # remaining chunks become filler work; chunk n is complete once
            # its last group (the v tile 4n+3) has been emitted
            for n in range(3, NBLK):
                for m in range(4):
                    filler_q.append((None, lambda m=m, n=n: qkv_group(m, n)))
                    filler_q.append(
                        (n if m == 3 else None, lambda t=4 * n + m: v_group(t))
                    )

            # ---- main pipeline ----
            for i in range(NBLK):
                drain_through_chunk(i)
                for h in range(HPC):
                    attn_block(h, i)
                pop_filler(2)
                flush_norms()
                for m in range(KC):
                    filler_q.append((None, lambda m=m, n=i: proj_group(m, n)))

            flush_norms()
            while filler_q:
                pop_filler(1)

    nc.compile()
    return nc


def _per_core_inputs(c, x, w_attn, b_attn, xt_cache):
    b, g = divmod(c, 4)
    hs = [HPC * g + j for j in range(HPC)]

    NPDT = FP8_NP if USE_FP8 else BF16_NP
    if b not in xt_cache:
        xt_cache[b] = np.ascontiguousarray(x[b].T).astype(NPDT)
    xt = xt_cache[b]

    qc = lambda h: w_attn[:, h * HD : (h + 1) * HD]
    kc = lambda h: w_attn[:, C + h * HD : C + (h + 1) * HD]
    vc = lambda h: w_attn[:, 2 * C + h * HD : 2 * C + (h + 1) * HD]
    z = np.zeros((C, HD), np.float32)
    wqk = (
        np.concatenate(
            [qc(hs[0]), qc(hs[1]), kc(hs[0]), kc(hs[1]), qc(hs[2]), z, kc(hs[2]), z],
            axis=1,
        )
        * WS
    ).astype(NPDT)
    wv = (np.concatenate([vc(h) for h in hs], axis=1) * WS).astype(NPDT)

    bq = lambda h: b_attn[h * HD : (h + 1) * HD]
    bk = lambda h: b_attn[C + h * HD : C + (h + 1) * HD]
    z64 = np.zeros(HD, np.float32)
    sc = 1.0 / np.sqrt(np.float32(HD))
    bias_qk = np.stack(
        [
            np.concatenate([bq(hs[0]), bq(hs[1])]) * sc,
            np.concatenate([bk(hs[0]), bk(hs[1])]),
            np.concatenate([bq(hs[2]) * sc, z64]),
            np.concatenate([bk(hs[2]), z64]),
        ],
        axis=1,
    ).astype(np.float32)
    # the 1/WS dequant of the fp8 weight prescale folds into the scales
    scale_qk = (
        np.stack(
            [
                np.full(P, sc),
                np.ones(P),
                np.concatenate([np.full(HD, sc), np.ones(HD)]),
                np.ones(P),
            ],
            axis=1,
        )
        / WS
    ).astype(np.float32)
    bv = np.broadcast_to(
        (
            np.concatenate(
                [b_attn[2 * C + h * HD : 2 * C + (h + 1) * HD] for h in hs]
            )
            * WS
        ).astype(np.float32)[None, :],
        (P, HPC * HD),
    ).copy()

    # multiplicative causal mask: keep k<=q (partition p = local k, col c = q)
    mask = (np.arange(P)[:, None] <= np.arange(P)[None, :]).astype(BF16_NP)

    return {
        "xt": xt,
        "wqk": wqk,
        "wv": wv,
        "wp": None,  # filled by caller (shared per group)
        "scale_qk": scale_qk,
        "bias_qk": bias_qk,
        "bv": bv,
        "mask": mask,
    }


def build_in_maps(x, w_attn, b_attn, w_proj):
    x = np.asarray(x, np.float32)
    w_attn = np.asarray(w_attn, np.float32)
    b_attn = np.asarray(b_attn, np.float32)
    w_proj = np.asarray(w_proj, np.float32)

    xt_cache = {}
    in_maps = []
    for c in range(NCORES):
        m = _per_core_inputs(c, x, w_attn, b_attn, xt_cache)
        g = c % 4
        hs = [HPC * g + j for j in range(HPC)]
        m["wp"] = np.stack(
            [w_proj[h * HD : (h + 1) * HD, :] for h in hs]
        ).astype(BF16_NP)
        in_maps.append(m)
    return in_maps


def kernel(x, w_attn, b_attn, w_proj, b_proj, _return_raw=False):
    x = np.asarray(x, np.float32)
    b_proj = np.asarray(b_proj, np.float32)

    if "nc" not in _CACHE:
        _CACHE["nc"] = _build_nc()
    nc = _CACHE["nc"]

    in_maps = build_in_maps(x, w_attn, b_attn, w_proj)
    res = run_bass_kernel_spmd(nc, in_maps, list(range(NCORES)))
    outs = [r["outT"] for r in res.results]

    full = np.empty((B, T, C), np.float32)
    for b in range(B):
        acc = outs[4 * b].astype(np.float32).copy()
        for g in range(1, 4):
            acc += outs[4 * b + g]
        full[b] = acc.T
    full += b_proj[None, None, :]
    if _return_raw:
        return full, res
    return full
